# revision 35
# baseline (speedup 1.0000x reference)
"""Multi-head causal attention (B=2, S=2048, D=2048, H=16) on 8 TRN2 NeuronCores.

Sharding (host-side): core c in 0..7 handles batch b=c//4 and heads
4*(c%4)..4*(c%4)+4 (a 512-wide column slice of wq/wk/wv, row slice of wp).
Each core computes its 4 heads' attention and a partial output projection
[S, D]; the host sums the 4 partials per batch and adds bp.

Per-core kernel (all matmuls in float32r -> full PE speed, ~2e-4 rel err):
  A) QKV projections from host-pre-transposed xT (d-major):
       qT/kT per head in [hd=128, S] layout, v per head in [S, hd] natural
       layout, written to DRAM scratch.
  B) Per head, per 512-wide q chunk j: scoresT = K_tile @ Q_chunk in
     [keys, q] layout (causal: only key tiles <= diagonal).  The causal mask
     for diagonal blocks is ADDED IN PSUM by a second matmul
     (identity^T @ mask_slice), keeping DVE off the critical path.  exp via
     ACT with the 1/sqrt(hd) scale folded in.  ctxT[hd, q] and the softmax
     denominators (ones^T @ pT partition sums) accumulate in PSUM over key
     tiles; reciprocal + ones-outer-product broadcast + one DVE multiply
     normalize into ctxT.
  C) Output projection: out[q, :] += ctxT_h^T @ wp_h accumulated over heads.
"""
import sys
if "/opt/trn_rl_repo" not in sys.path:
    sys.path.insert(0, "/opt/trn_rl_repo")

import numpy as np

B, S, D = 2, 2048, 2048
H, HD = 16, 128
NCORES = 8
HH = 4            # heads per core
CW = HH * HD      # 512 column slice per core
P = 128
KT = D // P       # 16 contraction tiles
NQC = 4           # q chunks of 512
NKT = S // P      # 16 key tiles
SCALE = 1.0 / float(np.sqrt(HD))
MASK_NEG = -1.0e10

_cache = {}


def _build():
    import concourse.bass as bass
    import concourse.tile as tile
    from concourse import bacc, mybir
    from concourse.masks import make_identity

    F32 = mybir.dt.float32
    F32R = mybir.dt.float32r
    AF = mybir.ActivationFunctionType
    ALU = mybir.AluOpType

    nc = bacc.Bacc("TRN2", target_bir_lowering=False, debug=False, num_devices=NCORES)

    xt = nc.dram_tensor("xt", [D, S], F32R, kind="ExternalInput")      # x[b].T
    wq = nc.dram_tensor("wq", [D, CW], F32R, kind="ExternalInput")
    wk = nc.dram_tensor("wk", [D, CW], F32R, kind="ExternalInput")
    wv = nc.dram_tensor("wv", [D, CW], F32R, kind="ExternalInput")
    wp = nc.dram_tensor("wp", [CW, D], F32R, kind="ExternalInput")
    # bq/bk arrive host-pre-arranged as [p, h] so the load is contiguous
    bq = nc.dram_tensor("bq", [P, HH], F32, kind="ExternalInput")
    bk = nc.dram_tensor("bk", [P, HH], F32, kind="ExternalInput")
    bv = nc.dram_tensor("bv", [CW], F32, kind="ExternalInput")
    out = nc.dram_tensor("out", [S, D], F32, kind="ExternalOutput")

    with tile.TileContext(nc) as tc:
        with tc.tile_pool(name="consts", bufs=1) as consts, \
             tc.tile_pool(name="dram", bufs=1, space="DRAM") as dram:
            # DRAM scratch: qT/kT [head][hd, S], v [head][S, hd] (contiguous
            # per-head so phase B loads are linear 64KB copies)
            qT_d = dram.tile([HH, P, S], F32R)
            v_d = dram.tile([HH, S, HD], F32R)

            # per-head per-partition biases for qT/kT layout: [p, h]
            # (SWDGE so the sync/scalar queues start on xt immediately)
            bq_sb = consts.tile([P, HH], F32)
            bk_sb = consts.tile([P, HH], F32)
            bv_sb = consts.tile([P, CW], F32)
            # ones vectors (fp32r) for denominator / broadcast matmuls
            ones_f32 = consts.tile([P, 1], F32)
            nc.vector.memset(ones_f32, 1.0)
            ones_col = consts.tile([P, 1], F32R)
            nc.vector.tensor_copy(ones_col, ones_f32)
            ones_row_f32 = consts.tile([1, P], F32)
            nc.vector.memset(ones_row_f32, 1.0)
            ones_row = consts.tile([1, P], F32R)
            nc.vector.tensor_copy(ones_row, ones_row_f32)
            # causal masks + identity built up front (values 0/-1e10/1 are
            # exact in any float width, so build straight into fp32r)
            mask_r = consts.tile([P, 896], F32R)
            nc.vector.memset(mask_r.bitcast(F32), 0.0)
            nc.gpsimd.affine_select(
                out=mask_r, in_=mask_r,
                compare_op=ALU.is_ge, fill=MASK_NEG,
                base=-384, channel_multiplier=-1, pattern=[[1, 896]],
            )
            ident_r = consts.tile([P, P], F32R)
            nc.vector.memset(ident_r.bitcast(F32), 0.0)
            nc.gpsimd.affine_select(
                out=ident_r, in_=ident_r,
                compare_op=ALU.not_equal, fill=1.0,
                base=0, channel_multiplier=1, pattern=[[-1, P]],
            )

            # kT for all heads stays in SBUF through phase B (saves the
            # DRAM round-trip and B-side reloads)
            kT_all = consts.tile([P, HH, S], F32R)

            # ---------------- Phase A: QKV projections ----------------
            with tc.tile_pool(name="xt_pool", bufs=2 * KT) as xt_pool, \
                 tc.tile_pool(name="w_pool", bufs=3 * KT) as w_pool, \
                 tc.tile_pool(name="stA", bufs=4) as stA, \
                 tc.tile_pool(name="psA", bufs=8, space="PSUM") as psA:

                HQ = [nc.sync, nc.scalar]
                # weights on SWDGE upfront; xt on the two HWDGE queues in
                # per-chunk [128, 512] tiles issued chunk-major so the first
                # q chunk's operands arrive first (the DMA fabric is a shared
                # serial resource -- JIT arrival order matters)
                w_ts = {}
                for wname, wdram in (("wq", wq), ("wk", wk)):
                    lst = []
                    for kt in range(KT):
                        t = w_pool.tile([P, CW], F32R, tag="w",
                                        name=f"{wname}_{kt}")
                        nc.gpsimd.dma_start(t, wdram[kt * P:(kt + 1) * P, :])
                        lst.append(t)
                    w_ts[wname] = lst
                    if wname == "wq":
                        nc.gpsimd.dma_start(bq_sb, bq[:])
                        nc.gpsimd.dma_start(bk_sb, bk[:])
                xt_t = [[None] * NQC for _ in range(KT)]

                def load_xt_chunk(c4):
                    for kt in range(KT):
                        t = xt_pool.tile([P, 512], F32R, tag="xt",
                                         name=f"xt{kt}_{c4}")
                        HQ[kt % 2].dma_start(
                            t, xt[kt * P:(kt + 1) * P, c4 * 512:(c4 + 1) * 512])
                        xt_t[kt][c4] = t

                load_xt_chunk(0)
                load_xt_chunk(1)
                # wv + bv on the scalar HWDGE queue: lands after the first two
                # xt chunks, before the first v sub-pass needs it (SWDGE
                # descriptor-gen would deliver it too late)
                lst = []
                for kt in range(KT):
                    t = w_pool.tile([P, CW], F32R, tag="w", name=f"wv_{kt}")
                    nc.scalar.dma_start(t, wv[kt * P:(kt + 1) * P, :])
                    lst.append(t)
                w_ts["wv"] = lst
                nc.scalar.dma_start(
                    bv_sb, bass.AP(tensor=bv, offset=0, ap=[[0, P], [1, CW]])
                )
                load_xt_chunk(2)
                load_xt_chunk(3)

                def a_qk(wname, c4):
                    bias_sb = bq_sb if wname == "wq" else bk_sb
                    scratch = qT_d if wname == "wq" else None
                    w_t = w_ts[wname]
                    pss = [psA.tile([P, 512], F32, tag="psA",
                                    name=f"psA{c4}_{h}") for h in range(HH)]
                    for kt in range(KT):
                        for h in range(HH):
                            nc.tensor.matmul(
                                pss[h],
                                w_t[kt][:, h * HD:(h + 1) * HD],
                                xt_t[kt][c4],
                                start=(kt == 0), stop=(kt == KT - 1),
                            )
                    for h in range(HH):
                        if scratch is None:
                            # kT: bias-add straight into resident SBUF
                            nc.scalar.activation(
                                kT_all[:, h, c4 * 512:(c4 + 1) * 512],
                                pss[h], AF.Identity,
                                bias=bias_sb[:, h:h + 1], scale=1.0,
                            )
                        else:
                            st = stA.tile([P, 512], F32R, tag="stA",
                                          name="stA_qk")
                            nc.scalar.activation(
                                st, pss[h], AF.Identity,
                                bias=bias_sb[:, h:h + 1], scale=1.0,
                            )
                            nc.gpsimd.dma_start(
                                scratch[h][:, c4 * 512:(c4 + 1) * 512], st
                            )

                def a_v(c4):
                    w_t = w_ts["wv"]
                    for st16 in range(4 * c4, 4 * c4 + 4):
                        psv = psA.tile([P, 512], F32, tag="psA",
                                       name=f"psV{st16}")
                        for kt in range(KT):
                            nc.tensor.matmul(
                                psv,
                                xt_t[kt][c4][:, (st16 % 4) * P:
                                             (st16 % 4 + 1) * P],
                                w_t[kt],
                                start=(kt == 0), stop=(kt == KT - 1),
                            )
                        st = stA.tile([P, 512], F32R, tag="stA", name="stA_v")
                        nc.vector.tensor_tensor(st, psv, bv_sb, ALU.add)
                        for h in range(HH):
                            HQ[(st16 + h) % 2].dma_start(
                                v_d[h][st16 * P:(st16 + 1) * P, :],
                                st[:, h * HD:(h + 1) * HD],
                            )

                # chunk-group order: xt chunk c4 dies after a_v(c4), so only
                # two chunks of xt are ever resident
                a_qk("wq", 0)
                a_qk("wq", 1)
                a_qk("wk", 0)
                a_v(0)
                a_qk("wq", 2)
                a_qk("wk", 1)
                a_v(1)
                a_qk("wq", 3)
                a_qk("wk", 2)
                a_v(2)
                a_qk("wk", 3)
                a_v(3)

            # ---------------- Phases B+C shared tiles ----------------
            with tc.tile_pool(name="bc_pool", bufs=1) as bc_pool:
                ctxT_sb = bc_pool.tile([P, HH, S], F32R)

                # ---------------- Phase B: attention ----------------
                with tc.tile_pool(name="qkv_pool", bufs=2) as qkv_pool, \
                     tc.tile_pool(name="vh_pool", bufs=2 * NKT) as vh_pool, \
                     tc.tile_pool(name="pT_pool", bufs=2 * NKT + 12) as pT_pool, \
                     tc.tile_pool(name="accB", bufs=2) as accB, \
                     tc.tile_pool(name="stB", bufs=2) as stB, \
                     tc.tile_pool(name="psS", bufs=4, space="PSUM") as psS, \
                     tc.tile_pool(name="psCtx", bufs=3, space="PSUM") as psCtx, \
                     tc.tile_pool(name="psT", bufs=1, space="PSUM") as psT, \
                     nc.allow_low_precision(
                         reason="float32r tiles are 4-byte fp32 containers; "
                                "PE rounds on read, DVE writes full fp32 bits"):

                    def b_scores(h, j, qT_sb):
                        # scoresT blocks + exp for q chunk j; diagonal blocks
                        # get the causal mask added in PSUM by a 2nd matmul
                        nkt = 4 * j + 4
                        qs = qT_sb[:, j * 512:(j + 1) * 512]
                        pt_t = []
                        for i in range(nkt):
                            ps_s = psS.tile([P, 512], F32, tag="ps_s")
                            m = i - 4 * j
                            nc.tensor.matmul(
                                ps_s, kT_all[:, h, i * P:(i + 1) * P], qs,
                                start=True, stop=(m < 0),
                            )
                            if m >= 0:
                                nc.tensor.matmul(
                                    ps_s, ident_r,
                                    mask_r[:, 384 - P * m:896 - P * m],
                                    start=False, stop=True,
                                )
                            pt = pT_pool.tile([P, 512], F32R, tag="pt",
                                              name=f"pt{h}_{j}_{i}")
                            nc.scalar.activation(pt, ps_s, AF.Exp, scale=SCALE)
                            pt_t.append(pt)
                        return pt_t

                    def b_tail(h, j, v_t, pt_t):
                        # ctxT and denominator PSUM accumulations, then
                        # normalize into ctxT_sb
                        nkt = 4 * j + 4
                        ps_c = psCtx.tile([P, 512], F32, tag="ps_c")
                        for i in range(nkt):
                            nc.tensor.matmul(
                                ps_c, v_t[i], pt_t[i],
                                start=(i == 0), stop=(i == nkt - 1),
                            )
                        ps_d = psT.tile([1, 512], F32, tag="ps_db")
                        for i in range(nkt):
                            nc.tensor.matmul(
                                ps_d, ones_col, pt_t[i],
                                start=(i == 0), stop=(i == nkt - 1),
                            )
                        rden = accB.tile([1, 512], F32R, tag="rden")
                        nc.vector.reciprocal(rden, ps_d)
                        ps_b = psT.tile([P, 512], F32, tag="ps_db")
                        nc.tensor.matmul(ps_b, ones_row, rden,
                                         start=True, stop=True)
                        rdenb = stB.tile([P, 512], F32, tag="rdenb")
                        nc.scalar.copy(rdenb, ps_b)
                        nc.vector.tensor_tensor(
                            ctxT_sb[:, h, j * 512:(j + 1) * 512],
                            ps_c, rdenb, ALU.mult,
                        )

                    HQ = [nc.sync, nc.scalar]

                    def load_head(h):
                        qT_sb = qkv_pool.tile([P, S], F32R, tag="qT",
                                              name=f"qT{h}")
                        nc.sync.dma_start(qT_sb, qT_d[h])
                        v_t = []
                        for i in range(NKT):
                            t = vh_pool.tile([P, HD], F32R, tag="vh",
                                             name=f"vh{h}_{i}")
                            HQ[i % 2].dma_start(
                                t, v_d[h][i * P:(i + 1) * P, :]
                            )
                            v_t.append(t)
                        return qT_sb, v_t

                    # tails lag scores by two chunks: the ACT exp stream of
                    # chunk j must finish before tail(j)'s last ctx matmul,
                    # so give PE two chunks of score work to chew in between
                    from collections import deque
                    pend = deque()
                    loaded = load_head(0)
                    for h in range(HH):
                        qT_sb, v_t = loaded
                        if h + 1 < HH:
                            loaded = load_head(h + 1)
                        for j in range(NQC):
                            pt_t = b_scores(h, j, qT_sb)
                            pend.append((h, j, v_t, pt_t))
                            if len(pend) > 1:
                                b_tail(*pend.popleft())
                    while pend:
                        b_tail(*pend.popleft())

                # ---------------- Phase C: output projection ----------------
                with tc.tile_pool(name="wp_pool", bufs=HH) as wp_pool, \
                     tc.tile_pool(name="outC", bufs=8) as outC, \
                     tc.tile_pool(name="psC", bufs=8, space="PSUM") as psC:
                    wp_t = []
                    for hh in range(HH):
                        t = wp_pool.tile([P, D], F32R, tag="wp", name=f"wp{hh}")
                        nc.gpsimd.dma_start(t, wp[hh * P:(hh + 1) * P, :])
                        wp_t.append(t)
                    for t16 in range(NKT):
                        for c4 in range(NQC):
                            ps_o = psC.tile([P, 512], F32, tag="psC",
                                            name=f"psC{t16}_{c4}")
                            for hh in range(HH):
                                nc.tensor.matmul(
                                    ps_o,
                                    ctxT_sb[:, hh, t16 * P:(t16 + 1) * P],
                                    wp_t[hh][:, c4 * 512:(c4 + 1) * 512],
                                    start=(hh == 0), stop=(hh == HH - 1),
                                )
                            o_st = outC.tile([P, 512], F32, tag="out",
                                             name=f"out{t16}_{c4}")
                            nc.any.tensor_copy(o_st, ps_o)
                            [nc.sync, nc.scalar][(t16 + c4) % 2].dma_start(
                                out[t16 * P:(t16 + 1) * P,
                                    c4 * 512:(c4 + 1) * 512], o_st)

    nc.compile()
    return nc


def _get_nc():
    if "nc" not in _cache:
        _cache["nc"] = _build()
    return _cache["nc"]


def _in_maps(x, wq, bq, wk, bk, wv, bv, wp):
    x = np.asarray(x, dtype=np.float32)
    maps = []
    xT = [np.ascontiguousarray(x[b].T) for b in range(B)]
    for c in range(NCORES):
        b = c // 4
        cols = slice((c % 4) * CW, (c % 4) * CW + CW)
        maps.append({
            "xt": xT[b],
            "wq": np.ascontiguousarray(np.asarray(wq, np.float32)[:, cols]),
            "wk": np.ascontiguousarray(np.asarray(wk, np.float32)[:, cols]),
            "wv": np.ascontiguousarray(np.asarray(wv, np.float32)[:, cols]),
            "wp": np.ascontiguousarray(np.asarray(wp, np.float32)[cols, :]),
            "bq": np.ascontiguousarray(
                np.asarray(bq, np.float32)[cols].reshape(HH, P).T),
            "bk": np.ascontiguousarray(
                np.asarray(bk, np.float32)[cols].reshape(HH, P).T),
            "bv": np.ascontiguousarray(np.asarray(bv, np.float32)[cols]),
        })
    return maps


def kernel(x, wq, bq, wk, bk, wv, bv, wp, bp):
    from concourse.bass_utils import run_bass_kernel_spmd

    nc = _get_nc()
    maps = _in_maps(x, wq, bq, wk, bk, wv, bv, wp)
    res = run_bass_kernel_spmd(nc, maps, core_ids=list(range(NCORES)))
    parts = [res.results[c]["out"] for c in range(NCORES)]
    bp = np.asarray(bp, dtype=np.float32)
    full = np.empty((B, S, D), dtype=np.float32)
    for b in range(B):
        acc = parts[4 * b].astype(np.float64)
        for c in range(4 * b + 1, 4 * b + 4):
            acc += parts[c]
        full[b] = (acc + bp).astype(np.float32)
    return full
